# revision 32
# baseline (speedup 1.0000x reference)
"""Trainium2 Bass kernel for DetectionConfidenceMap2keypoint (3kp).

Computes, for two heatmap stacks Rk / tf_Rk of shape [16, 64, 96, 96]:
  D = sigmoid(R)                                    (full-size output)
  zeta = sum_{h,w} D,  kx = sum w*D,  ky = sum h*D  (per (b,k))
on 8 NeuronCores (batch sharded, 2 batches/core -> 128 (b,k) maps/core,
one map per SBUF partition). The tiny O(B*K) soft-argmax decode
(round/gather/trunc) runs on host from the device results.

Pipeline per free-dim chunk (4 chunks of 24 h-rows per stack):
  sync-seq:  DMA in
  ScalarE:   sigmoid + accum_out -> zeta partial, then DMA out (same
             sequencer, so the data-ready wait is free by program order)
  VectorE:   row sums (X-reduce) + lower colsum tree folds
  GpSimd:    first colsum tree fold (contiguous adds over h rows)
"""

import sys

import numpy as np

if "/opt/trn_rl_repo" not in sys.path:
    sys.path.insert(0, "/opt/trn_rl_repo")

B, K, H, W = 16, 64, 96, 96
N_CORES = 8
B_SH = B // N_CORES          # batches per core
P = B_SH * K                 # 128 partition maps per core per stack
FREE = H * W                 # 9216

# h-rows per chunk per stack; each list sums to 96
CHUNK_ROWS_PER_STACK = [[24, 24, 24, 24], [24, 24, 24, 24]]

_NC_CACHE = {}


def _colsum_tree(nc, alu, treepool, f32, src, rows, dst):
    """dst[:, 0:96] = sum over `rows` h-rows of src (contiguous adds).

    First fold runs on GpSimd, the rest on VectorE, splitting the
    elementwise-add work across the two otherwise-idle engines.
    """
    engines = [nc.gpsimd] + [nc.vector] * 10
    ei = 0
    cur, cur_rows = src, rows
    while cur_rows > 3 and cur_rows % 2 == 0:
        half = cur_rows // 2
        t = treepool.tile([P, half * W], f32, tag=f"fold{half}")
        engines[ei].tensor_tensor(t[:], cur[:, 0:half * W],
                                  cur[:, half * W:cur_rows * W], op=alu.add)
        ei += 1
        cur, cur_rows = t, half
    if cur_rows == 3:
        t = treepool.tile([P, W], f32, tag="fold_pair")
        nc.vector.tensor_tensor(t[:], cur[:, 0:W], cur[:, W:2 * W], op=alu.add)
        nc.vector.tensor_tensor(dst, t[:], cur[:, 2 * W:3 * W], op=alu.add)
    else:
        assert cur_rows == 2
        nc.vector.tensor_tensor(dst, cur[:, 0:W], cur[:, W:2 * W], op=alu.add)


def _build_nc():
    import concourse.tile as tile
    from concourse import bacc, mybir

    f32 = mybir.dt.float32
    act = mybir.ActivationFunctionType
    alu = mybir.AluOpType
    ax = mybir.AxisListType

    nc = bacc.Bacc("TRN2", target_bir_lowering=False)

    r = nc.dram_tensor("r", [P, FREE], f32, kind="ExternalInput")
    tr = nc.dram_tensor("tr", [P, FREE], f32, kind="ExternalInput")
    d = nc.dram_tensor("d", [P, FREE], f32, kind="ExternalOutput")
    td = nc.dram_tensor("td", [P, FREE], f32, kind="ExternalOutput")
    stats = nc.dram_tensor("stats", [P, 6], f32, kind="ExternalOutput")

    with tile.TileContext(nc) as tc:
        with (
            tc.tile_pool(name="consts", bufs=1) as cpool,
            tc.tile_pool(name="tin", bufs=6) as tinpool,
            tc.tile_pool(name="tout", bufs=10) as toutpool,
            tc.tile_pool(name="tree", bufs=3) as treepool,
            tc.tile_pool(name="acc", bufs=1) as apool,
        ):
            # arange(0..95) per partition; weights for both kx and ky finals
            wv = cpool.tile([P, W], f32)
            nc.gpsimd.iota(wv[:], [[1, W]], base=0, channel_multiplier=0,
                           allow_small_or_imprecise_dtypes=True)

            stats_sb = apool.tile([P, 6], f32)

            for si, (rin, dout) in enumerate(((r, d), (tr, td))):
                chunk_rows = CHUNK_ROWS_PER_STACK[si]
                assert sum(chunk_rows) == H
                nch = len(chunk_rows)
                starts = np.cumsum([0] + chunk_rows[:-1]).tolist()
                zp = apool.tile([P, nch], f32, tag=f"zp{si}")
                rows = apool.tile([P, H], f32, tag=f"rows{si}")        # rowsum
                colp = apool.tile([P, nch * W], f32, tag=f"colp{si}")  # colsum partials
                for c, (r0, rcnt) in enumerate(zip(starts, chunk_rows)):
                    ch = rcnt * W
                    sl = slice(r0 * W, r0 * W + ch)
                    tin = tinpool.tile([P, ch], f32, tag="tin")
                    # chunk 1 of stack 0 is issued from the scalar sequencer:
                    # DIRECT2D descriptor distribution is ~27ns/row serial per
                    # sequencer, so overlapping the first two transfers' gens
                    # shortens the pipeline-fill ramp (ACT is idle until the
                    # first sigmoid anyway)
                    in_eng = nc.scalar if (si == 0 and c == 1) else nc.sync
                    in_eng.dma_start(tin[:], rin[:, sl])
                    tout = toutpool.tile([P, ch], f32, tag="tout")
                    nc.scalar.activation(tout[:], tin[:], act.Sigmoid,
                                         accum_out=zp[:, c:c + 1])
                    # out-DMA on the scalar-engine sequencer: the producing
                    # sigmoid precedes it there, so its wait is satisfied by
                    # program order and can't head-of-line-block in-DMAs
                    nc.scalar.dma_start(dout[:, sl], tout[:])
                    # row sums on VectorE (contiguous X-reduce)
                    v3 = tout[:].rearrange("p (h w) -> p h w", h=rcnt, w=W)
                    nc.vector.reduce_sum(rows[:, r0:r0 + rcnt], v3, axis=ax.X)
                    # col sums via contiguous add-tree (GpSimd + VectorE)
                    _colsum_tree(nc, alu, treepool, f32, tout,
                                 rcnt, colp[:, c * W:(c + 1) * W])

                base = si * 3
                # zeta
                nc.vector.reduce_sum(stats_sb[:, base:base + 1], zp[:],
                                     axis=ax.X)
                # colsum over the chunk partials: view [p, w, c], reduce c
                cols = apool.tile([P, W], f32, tag=f"cols{si}")
                cpv = colp[:].rearrange("p (c w) -> p w c", c=nch, w=W)
                nc.vector.reduce_sum(cols[:], cpv, axis=ax.X)
                # kx = sum_w w * colsum
                wcol = apool.tile([P, W], f32, tag=f"wcol{si}")
                nc.vector.tensor_tensor(wcol[:], cols[:], wv[:], op=alu.mult)
                nc.vector.reduce_sum(stats_sb[:, base + 1:base + 2], wcol[:],
                                     axis=ax.X)
                # ky = sum_h h * rowsum
                wrow = apool.tile([P, H], f32, tag=f"wrow{si}")
                nc.vector.tensor_tensor(wrow[:], rows[:], wv[:], op=alu.mult)
                nc.vector.reduce_sum(stats_sb[:, base + 2:base + 3], wrow[:],
                                     axis=ax.X)
                # ship this stack's stats as soon as its finals are done
                nc.sync.dma_start(stats[:, base:base + 3],
                                  stats_sb[:, base:base + 3])

    nc.finalize()
    return nc


def _get_nc():
    if "nc" not in _NC_CACHE:
        _NC_CACHE["nc"] = _build_nc()
    return _NC_CACHE["nc"]


def _decode_host(Dk, zeta, kx, ky):
    # all float32, mirrors reference._decode
    kpx = np.round(kx / zeta)
    kpy = np.round(ky / zeta)
    kp = np.stack([kpx, kpy], axis=2)                     # [B, K, 2]
    w_idx = kpx.astype(np.int32)
    h_idx = kpy.astype(np.int32)
    b_idx = np.arange(Dk.shape[0])[:, None]
    k_idx = np.arange(Dk.shape[1])[None, :]
    dv = Dk[b_idx, k_idx, h_idx, w_idx]                   # [B, K]
    kp1 = np.trunc(kp + kp * dv[..., None])
    kp2 = np.trunc(kp - kp * dv[..., None])
    return np.concatenate([kp, kp1, kp2], axis=1)         # [B, 3K, 2]


def kernel(Rk, tf_Rk, my_height, my_width, **_kw):
    from concourse.bass_utils import run_bass_kernel_spmd

    assert int(my_height) == H and int(my_width) == W
    Rk = np.ascontiguousarray(np.asarray(Rk, dtype=np.float32))
    tf_Rk = np.ascontiguousarray(np.asarray(tf_Rk, dtype=np.float32))

    in_maps = []
    for i in range(N_CORES):
        bsl = slice(i * B_SH, (i + 1) * B_SH)
        in_maps.append({
            "r": Rk[bsl].reshape(P, FREE),
            "tr": tf_Rk[bsl].reshape(P, FREE),
        })

    res = run_bass_kernel_spmd(_get_nc(), in_maps, core_ids=list(range(N_CORES)))

    Dk = np.empty((B, K, H, W), dtype=np.float32)
    tf_Dk = np.empty((B, K, H, W), dtype=np.float32)
    st = np.empty((B, K, 6), dtype=np.float32)
    for i, out in enumerate(res.results):
        bsl = slice(i * B_SH, (i + 1) * B_SH)
        Dk[bsl] = out["d"].reshape(B_SH, K, H, W)
        tf_Dk[bsl] = out["td"].reshape(B_SH, K, H, W)
        st[bsl] = out["stats"].reshape(B_SH, K, 6)

    zeta, kx, ky = st[..., 0], st[..., 1], st[..., 2]
    t_zeta, t_kx, t_ky = st[..., 3], st[..., 4], st[..., 5]

    keypoint = _decode_host(Dk, zeta, kx, ky)
    tf_keypoint = _decode_host(tf_Dk, t_zeta, t_kx, t_ky)

    return (Dk, tf_Dk, keypoint, tf_keypoint,
            np.ascontiguousarray(zeta), np.ascontiguousarray(t_zeta))


# revision 34
# speedup vs baseline: 1.1022x; 1.1022x over previous
"""Trainium2 Bass kernel for DetectionConfidenceMap2keypoint (3kp).

Computes, for two heatmap stacks Rk / tf_Rk of shape [16, 64, 96, 96]:
  D = sigmoid(R)                                    (full-size output)
  zeta = sum_{h,w} D,  kx = sum w*D,  ky = sum h*D  (per (b,k))
on 8 NeuronCores (batch sharded, 2 batches/core -> 128 (b,k) maps/core,
one map per SBUF partition). The tiny O(B*K) soft-argmax decode
(round/gather/trunc) runs on host from the device results.

Pipeline per free-dim chunk (4 chunks of 24 h-rows per stack):
  sync-seq:  DMA in
  ScalarE:   sigmoid + accum_out -> zeta partial, then DMA out (same
             sequencer, so the data-ready wait is free by program order)
  VectorE:   row sums (X-reduce) + lower colsum tree folds
  GpSimd:    first colsum tree fold (contiguous adds over h rows)
"""

import sys

import numpy as np

if "/opt/trn_rl_repo" not in sys.path:
    sys.path.insert(0, "/opt/trn_rl_repo")

B, K, H, W = 16, 64, 96, 96
N_CORES = 8
B_SH = B // N_CORES          # batches per core
P = B_SH * K                 # 128 partition maps per core per stack
FREE = H * W                 # 9216

# h-rows per chunk per stack; each list sums to 96
CHUNK_ROWS_PER_STACK = [[24, 24, 24, 24], [24, 24, 24, 12, 12]]

_NC_CACHE = {}


def _colsum_tree(nc, alu, treepool, f32, src, rows, dst):
    """dst[:, 0:96] = sum over `rows` h-rows of src (contiguous adds).

    First fold runs on GpSimd, the rest on VectorE, splitting the
    elementwise-add work across the two otherwise-idle engines.
    """
    engines = [nc.gpsimd] + [nc.vector] * 10
    ei = 0
    cur, cur_rows = src, rows
    while cur_rows > 3 and cur_rows % 2 == 0:
        half = cur_rows // 2
        t = treepool.tile([P, half * W], f32, tag=f"fold{half}")
        engines[ei].tensor_tensor(t[:], cur[:, 0:half * W],
                                  cur[:, half * W:cur_rows * W], op=alu.add)
        ei += 1
        cur, cur_rows = t, half
    if cur_rows == 3:
        t = treepool.tile([P, W], f32, tag="fold_pair")
        nc.vector.tensor_tensor(t[:], cur[:, 0:W], cur[:, W:2 * W], op=alu.add)
        nc.vector.tensor_tensor(dst, t[:], cur[:, 2 * W:3 * W], op=alu.add)
    else:
        assert cur_rows == 2
        nc.vector.tensor_tensor(dst, cur[:, 0:W], cur[:, W:2 * W], op=alu.add)


def _build_nc():
    import concourse.tile as tile
    from concourse import bacc, mybir

    f32 = mybir.dt.float32
    act = mybir.ActivationFunctionType
    alu = mybir.AluOpType
    ax = mybir.AxisListType

    nc = bacc.Bacc("TRN2", target_bir_lowering=False)

    r = nc.dram_tensor("r", [P, FREE], f32, kind="ExternalInput")
    tr = nc.dram_tensor("tr", [P, FREE], f32, kind="ExternalInput")
    d = nc.dram_tensor("d", [P, FREE], f32, kind="ExternalOutput")
    td = nc.dram_tensor("td", [P, FREE], f32, kind="ExternalOutput")
    stats = nc.dram_tensor("stats", [P, 6], f32, kind="ExternalOutput")

    with tile.TileContext(nc) as tc:
        with (
            tc.tile_pool(name="consts", bufs=1) as cpool,
            tc.tile_pool(name="tin", bufs=6) as tinpool,
            tc.tile_pool(name="tout", bufs=10) as toutpool,
            tc.tile_pool(name="tree", bufs=3) as treepool,
            tc.tile_pool(name="acc", bufs=1) as apool,
        ):
            # arange(0..95) per partition; weights for both kx and ky finals
            wv = cpool.tile([P, W], f32)
            nc.gpsimd.iota(wv[:], [[1, W]], base=0, channel_multiplier=0,
                           allow_small_or_imprecise_dtypes=True)

            stats_sb = apool.tile([P, 6], f32)

            for si, (rin, dout) in enumerate(((r, d), (tr, td))):
                chunk_rows = CHUNK_ROWS_PER_STACK[si]
                assert sum(chunk_rows) == H
                nch = len(chunk_rows)
                starts = np.cumsum([0] + chunk_rows[:-1]).tolist()
                zp = apool.tile([P, nch], f32, tag=f"zp{si}")
                rows = apool.tile([P, H], f32, tag=f"rows{si}")        # rowsum
                colp = apool.tile([P, nch * W], f32, tag=f"colp{si}")  # colsum partials
                for c, (r0, rcnt) in enumerate(zip(starts, chunk_rows)):
                    ch = rcnt * W
                    sl = slice(r0 * W, r0 * W + ch)
                    tin = tinpool.tile([P, ch], f32, tag="tin")
                    nc.sync.dma_start(tin[:], rin[:, sl])
                    tout = toutpool.tile([P, ch], f32, tag="tout")
                    nc.scalar.activation(tout[:], tin[:], act.Sigmoid,
                                         accum_out=zp[:, c:c + 1])
                    # out-DMA on the scalar-engine sequencer: the producing
                    # sigmoid precedes it there, so its wait is satisfied by
                    # program order and can't head-of-line-block in-DMAs
                    nc.scalar.dma_start(dout[:, sl], tout[:])
                    # row sums on VectorE (contiguous X-reduce)
                    v3 = tout[:].rearrange("p (h w) -> p h w", h=rcnt, w=W)
                    nc.vector.reduce_sum(rows[:, r0:r0 + rcnt], v3, axis=ax.X)
                    # col sums via contiguous add-tree (GpSimd + VectorE)
                    _colsum_tree(nc, alu, treepool, f32, tout,
                                 rcnt, colp[:, c * W:(c + 1) * W])

                base = si * 3
                # zeta
                nc.vector.reduce_sum(stats_sb[:, base:base + 1], zp[:],
                                     axis=ax.X)
                # colsum over the chunk partials: view [p, w, c], reduce c
                cols = apool.tile([P, W], f32, tag=f"cols{si}")
                cpv = colp[:].rearrange("p (c w) -> p w c", c=nch, w=W)
                nc.vector.reduce_sum(cols[:], cpv, axis=ax.X)
                # kx = sum_w w * colsum
                wcol = apool.tile([P, W], f32, tag=f"wcol{si}")
                nc.vector.tensor_tensor(wcol[:], cols[:], wv[:], op=alu.mult)
                nc.vector.reduce_sum(stats_sb[:, base + 1:base + 2], wcol[:],
                                     axis=ax.X)
                # ky = sum_h h * rowsum
                wrow = apool.tile([P, H], f32, tag=f"wrow{si}")
                nc.vector.tensor_tensor(wrow[:], rows[:], wv[:], op=alu.mult)
                nc.vector.reduce_sum(stats_sb[:, base + 2:base + 3], wrow[:],
                                     axis=ax.X)
                # ship this stack's stats as soon as its finals are done
                nc.sync.dma_start(stats[:, base:base + 3],
                                  stats_sb[:, base:base + 3])

    nc.finalize()
    return nc


def _get_nc():
    if "nc" not in _NC_CACHE:
        _NC_CACHE["nc"] = _build_nc()
    return _NC_CACHE["nc"]


def _decode_host(Dk, zeta, kx, ky):
    # all float32, mirrors reference._decode
    kpx = np.round(kx / zeta)
    kpy = np.round(ky / zeta)
    kp = np.stack([kpx, kpy], axis=2)                     # [B, K, 2]
    w_idx = kpx.astype(np.int32)
    h_idx = kpy.astype(np.int32)
    b_idx = np.arange(Dk.shape[0])[:, None]
    k_idx = np.arange(Dk.shape[1])[None, :]
    dv = Dk[b_idx, k_idx, h_idx, w_idx]                   # [B, K]
    kp1 = np.trunc(kp + kp * dv[..., None])
    kp2 = np.trunc(kp - kp * dv[..., None])
    return np.concatenate([kp, kp1, kp2], axis=1)         # [B, 3K, 2]


def kernel(Rk, tf_Rk, my_height, my_width, **_kw):
    from concourse.bass_utils import run_bass_kernel_spmd

    assert int(my_height) == H and int(my_width) == W
    Rk = np.ascontiguousarray(np.asarray(Rk, dtype=np.float32))
    tf_Rk = np.ascontiguousarray(np.asarray(tf_Rk, dtype=np.float32))

    in_maps = []
    for i in range(N_CORES):
        bsl = slice(i * B_SH, (i + 1) * B_SH)
        in_maps.append({
            "r": Rk[bsl].reshape(P, FREE),
            "tr": tf_Rk[bsl].reshape(P, FREE),
        })

    res = run_bass_kernel_spmd(_get_nc(), in_maps, core_ids=list(range(N_CORES)))

    Dk = np.empty((B, K, H, W), dtype=np.float32)
    tf_Dk = np.empty((B, K, H, W), dtype=np.float32)
    st = np.empty((B, K, 6), dtype=np.float32)
    for i, out in enumerate(res.results):
        bsl = slice(i * B_SH, (i + 1) * B_SH)
        Dk[bsl] = out["d"].reshape(B_SH, K, H, W)
        tf_Dk[bsl] = out["td"].reshape(B_SH, K, H, W)
        st[bsl] = out["stats"].reshape(B_SH, K, 6)

    zeta, kx, ky = st[..., 0], st[..., 1], st[..., 2]
    t_zeta, t_kx, t_ky = st[..., 3], st[..., 4], st[..., 5]

    keypoint = _decode_host(Dk, zeta, kx, ky)
    tf_keypoint = _decode_host(tf_Dk, t_zeta, t_kx, t_ky)

    return (Dk, tf_Dk, keypoint, tf_keypoint,
            np.ascontiguousarray(zeta), np.ascontiguousarray(t_zeta))
